# revision 8
# baseline (speedup 1.0000x reference)
"""Trainium2 Bass kernel for nn_Coords2RMSD (masked Kabsch RMSD loss).

Pure data parallel over 8 NeuronCores. Samples are globally sorted by
num_atoms and dealt round-robin to cores, so all cores share one
compiled schedule: per group of 16 samples, only ceil(max_n/128) atom
chunks are streamed/contracted, and at most the last chunk or two need
a per-sample mask. All 17 per-sample reductions (3x3 correlation,
component sums, sums of squares) are computed on the TensorEngine as a
batched 7x7 Gram matrix in fp8e4m3 (tolerance allows it; fp32 PSUM
accumulate): Z = [X1 X2 X3 Y1 Y2 Y3 one] columns, G = Z^T (mask*Z),
masking only the moving operand (the mask is idempotent). The whole
input stream is DMA'd up-front into persistent SBUF tiles across two
HWDGE rings so the TensorEngine free-runs. Per-sample 7x7 diagonal
blocks are gathered sample-major via a through-DRAM DMA shuffle, and a
closed-form 3x3 eigenvalue epilogue (trig method) turns the reductions
into the RMSD; extraction + epilogue run in two halves so the first
half hides under the main loop.
"""
import math
import numpy as np

P = 128          # partitions
M = 768          # max atoms
NCORES = 8
T = 8            # epilogue tiles (sample p of tile t is position 128t+p)
S = 1024         # samples per core
G = 64           # sample groups per core (16 samples each)
W = 112          # matmul columns per group (16 samples x 7 comps)
NSUP = 16        # xz stream load chunks (4 groups each)

_CACHE = {}


def _plan(na):
    """Global sort + deal; schedule shared by all cores."""
    na = np.asarray(na).astype(np.int64)
    order = np.argsort(na, kind="stable")
    n_pos = na[order].reshape(S, NCORES)      # [position, core]
    ngrp = n_pos.reshape(G, 16, NCORES)
    gmax = ngrp.max(axis=(1, 2))
    gmin = ngrp.min(axis=(1, 2))
    chunks = np.ceil(gmax / 128.0).astype(int)
    cmin = np.minimum(np.floor(gmin / 128.0).astype(int), chunks)
    nmask = chunks - cmin
    return order, tuple(int(c) for c in chunks), tuple(int(m) for m in nmask)


def _build(schedule):
    import concourse.bacc as bacc
    import concourse.mybir as mybir
    from concourse.tile import TileContext
    from concourse.hw_specs import get_activation_tables

    f32 = mybir.dt.float32
    fp8 = mybir.dt.float8e4
    ALU = mybir.AluOpType
    AF = mybir.ActivationFunctionType

    chunks, nmask = schedule
    Tn = T
    gw = [112 * c + 16 for c in chunks]          # stream width per group
    goff = np.concatenate([[0], np.cumsum(gw)]).astype(int)
    W_TOT = int(goff[-1])
    moff = np.concatenate([[0], np.cumsum([16 * m for m in nmask])]).astype(int)
    MK_TOT = max(int(moff[-1]), 16)

    nc = bacc.Bacc()
    xzd = nc.declare_dram_parameter("xz", [P, W_TOT], fp8, isOutput=False)
    mskd = nc.declare_dram_parameter("msk", [P, MK_TOT], fp8, isOutput=False)
    constsd = nc.declare_dram_parameter("consts", [P, 2 * Tn], f32,
                                        isOutput=False)
    outd = nc.declare_dram_parameter("out", [P, Tn], f32, isOutput=True)
    # per-half scratch for the diagonal-block gather: [s(16), g(32), ab(49)]
    scrd = [nc.dram_tensor(f"scr{h}", [16, G // 2, 49], f32, kind="Internal")
            for h in range(2)]

    with TileContext(nc) as tc:
        with tc.tile_pool(name="big", bufs=1) as big, \
             tc.tile_pool(name="wk", bufs=8) as wk, \
             tc.tile_pool(name="ps", bufs=8, space="PSUM") as ps, \
             tc.tile_pool(name="st", bufs=1) as st:
            consts_t = big.tile([P, 2 * Tn], f32)
            nc.sync.dma_start(out=consts_t[:], in_=constsd[:])
            invn_t = consts_t[:, Tn:2 * Tn]

            msk_t = big.tile([P, MK_TOT], fp8)
            nc.sync.dma_start(out=msk_t[:], in_=mskd[:])

            # whole xz stream resident in SBUF; alternate HWDGE rings
            xz_sup = []
            sup_bounds = []
            gps = G // NSUP
            for j in range(NSUP):
                lo, hi = int(goff[4 * j]), int(goff[4 * (j + 1)])
                xt = big.tile([P, hi - lo], fp8, tag=f"xz{j}")
                eng = nc.sync if j % 2 == 0 else nc.scalar
                eng.dma_start(out=xt[:], in_=xzd[:, lo:hi])
                xz_sup.append(xt)
                sup_bounds.append(lo)

            # Gram results per half: [112, 32*112] f32
            E_h = [big.tile([W, (G // 2) * W], f32, tag=f"ea{h}", name=f"ea{h}")
                   for h in range(2)]

            for g in range(G):
                ch, nm = chunks[g], nmask[g]
                cmin = ch - nm
                j = g // gps
                base = int(goff[g]) - sup_bounds[j]
                xg = xz_sup[j][:, base:base + gw[g]]

                rm = None
                if nm > 0:
                    rm = wk.tile([P, 112 * nm], fp8, tag="rm")
                    mg = msk_t[:, int(moff[g]):int(moff[g]) + 16 * nm]
                    nc.vector.tensor_tensor(
                        out=rm[:].rearrange("p (c s k) -> p c s k", c=nm, s=16),
                        in0=xg[:, 112 * cmin:112 * ch].rearrange(
                            "p (c s k) -> p c s k", c=nm, s=16),
                        in1=mg.rearrange("p (c s) -> p c s", c=nm)
                            .unsqueeze(3).broadcast_to([P, nm, 16, 7]),
                        op=ALU.mult)

                psum_t = ps.tile([P, W], f32, tag="ps")
                for c in range(ch):
                    # lhsT padded to 128 cols (pad cols only write psum
                    # rows 112-127, never read; host zeros the tail pad)
                    lhs = xg[:, W * c:W * c + 128]
                    if c < cmin:
                        rhs = xg[:, W * c:W * c + W]
                    else:
                        rhs = rm[:, W * (c - cmin):W * (c - cmin) + W]
                    nc.tensor.matmul(psum_t[:], lhsT=lhs, rhs=rhs,
                                     start=(c == 0), stop=(c == ch - 1))

                gh = g % (G // 2)
                nc.scalar.activation(out=E_h[g // (G // 2)][:, W * gh:W * (gh + 1)],
                                     in_=psum_t[0:W, :], func=AF.Copy)

            # ---------------- extraction + epilogue, per half ----------
            cnt = [0]

            def new(shape, nfree=None):
                cnt[0] += 1
                free = int(np.prod(shape[1:]))
                r = st.tile([P, free], f32, tag=f"e{cnt[0]}")
                ap = r[:]
                if len(shape) > 2:
                    names = " ".join(f"d{i}" for i in range(len(shape) - 1))
                    ap = ap.rearrange(f"p ({names}) -> p {names}",
                                      **{f"d{i}": int(shape[1 + i])
                                         for i in range(len(shape) - 1)})
                return ap

            def tt(a, b, op, shape=None, eng=None):
                r = new(list(shape or a.shape))
                (eng or nc.vector).tensor_tensor(out=r, in0=a, in1=b, op=op)
                return r

            def ts(a, s1, op0, s2=None, op1=None, eng=None):
                r = new(list(a.shape))
                if op1 is None:
                    (eng or nc.vector).tensor_scalar(out=r, in0=a, scalar1=s1,
                                                     scalar2=None, op0=op0)
                else:
                    (eng or nc.vector).tensor_scalar(out=r, in0=a, scalar1=s1,
                                                     scalar2=s2, op0=op0,
                                                     op1=op1)
                return r

            def stt(a, s, b, op0, op1, eng=None):
                r = new(list(a.shape))
                (eng or nc.vector).scalar_tensor_tensor(out=r, in0=a, scalar=s,
                                                        in1=b, op0=op0, op1=op1)
                return r

            def act(a, func, scale=1.0, bias=0.0):
                r = new(list(a.shape))
                nc.scalar.activation(out=r, in_=a, func=func,
                                     scale=scale, bias=bias)
                return r

            def recip(a):
                r = new(list(a.shape))
                nc.vector.reciprocal(out=r, in_=a)
                return r

            def red_inner(a, n_keep, eng=None):
                r = new([P, n_keep])
                (eng or nc.vector).tensor_reduce(out=r, in_=a,
                                                 axis=mybir.AxisListType.X,
                                                 op=ALU.add)
                return r

            def poly_eval(x, coeffs):
                g_ = ts(x, coeffs[0], ALU.mult)
                for c in coeffs[1:-1]:
                    g_ = stt(g_, c, x, ALU.add, ALU.mult)
                return ts(g_, coeffs[-1], ALU.add)

            hopb_engs = [nc.sync, nc.gpsimd]

            for h in range(2):
                Th = Tn // 2
                # hop B: E_h[7s+a, 112g + 7s+b] -> scr[s, g, 7a+b]
                for s in range(16):
                    src = E_h[h][7 * s:7 * s + 7, :].rearrange(
                        "p (g c) -> p g c", g=G // 2)[:, :, 7 * s:7 * s + 7]
                    dst = scrd[h][s].rearrange("g (a b) -> a g b", a=7)
                    hopb_engs[s % 2].dma_start(out=dst, in_=src)
                # hop C: scr[s, 8t+g8, ab] -> ep[16 g8 + s, 49 t + ab]
                ep = big.tile([P, Th * 49], f32, tag=f"ep{h}")
                nc.sync.dma_start(
                    out=ep[:],
                    in_=scrd[h][:].rearrange("s (t g8) ab -> g8 s t ab", t=Th))

                ep_r = ep[:].rearrange("p (t a b) -> p a b t", t=Th, a=7, b=7)
                mmv = ep_r[:, 0:3, 3:6, :]      # [P, i, j, Th] = sum Xi*Yj
                sxv = ep_r[:, 0:3, 6, :]
                syv = ep_r[:, 3:6, 6, :]
                ep_v = ep[:].rearrange("p (t e) -> p t e", t=Th)
                invn_h = invn_t[:, Th * h:Th * (h + 1)]

                # ssx = sum_i G[i,i], ssy = sum_j G[3+j,3+j]
                ssx = red_inner(ep_v[:, :, 0:17:8], Th)
                ssy = red_inner(ep_v[:, :, 24:41:8], Th)

                invn_b3 = invn_h.unsqueeze(1).broadcast_to([P, 3, Th])

                # R_ij = m_ij - (sx_i * invn) * sy_j
                meanx = tt(sxv, invn_b3, ALU.mult)
                meanx_v = meanx.unsqueeze(2).broadcast_to([P, 3, 3, Th])
                sy_v = syv.unsqueeze(1).broadcast_to([P, 3, 3, Th])
                mxsy = tt(meanx_v, sy_v, ALU.mult)
                Rv = tt(mmv, mxsy, ALU.subtract, shape=[P, 3, 3, Th])

                # e0 = ssx + ssy - (|sx|^2 + |sy|^2) * invn  (GpSimd)
                gp = nc.gpsimd
                sx2 = tt(sxv, sxv, ALU.mult, shape=[P, 3, Th], eng=gp)
                sy2 = tt(syv, syv, ALU.mult, shape=[P, 3, Th], eng=gp)
                nrm = tt(sx2, sy2, ALU.add, eng=gp)
                nrms = red_inner(nrm.rearrange("p i t -> p t i"), Th)
                ss = tt(ssx, ssy, ALU.add, eng=gp)
                nrmi = tt(nrms, invn_h, ALU.mult, eng=gp)
                e0 = tt(ss, nrmi, ALU.subtract, eng=gp)              # [P,Th]

                # A = R^T R (batched outer products over k)
                Av = new([P, 3, 3, Th])
                for k in range(3):
                    rk = Rv[:, k]
                    rk_a = rk.unsqueeze(2).broadcast_to([P, 3, 3, Th])
                    rk_b = rk.unsqueeze(1).broadcast_to([P, 3, 3, Th])
                    if k == 0:
                        nc.vector.tensor_tensor(out=Av, in0=rk_a, in1=rk_b,
                                                op=ALU.mult)
                    else:
                        pk = tt(rk_a, rk_b, ALU.mult)
                        nc.vector.tensor_tensor(out=Av, in0=Av, in1=pk,
                                                op=ALU.add)
                Aflat = Av.rearrange("p a b t -> p (a b) t")
                Adiag = Aflat[:, ::4]                                # [P,3,Th]

                q = ts(red_inner(Adiag.rearrange("p a t -> p t a"), Th),
                       1.0 / 3.0, ALU.mult)                          # [P,Th]
                q_b3 = q.unsqueeze(1).broadcast_to([P, 3, Th])

                # p2 = sum(B^2) = sum(A^2) - 3 q^2   (B = A - q I, tr A = 3q)
                asq = tt(Aflat, Aflat, ALU.mult)
                allsq = red_inner(asq.rearrange("p a t -> p t a"), Th)
                qsq = tt(q, q, ALU.mult)
                p2 = stt(qsq, -3.0, allsq, ALU.mult, ALU.add)        # [P,Th]

                # log-space: p = (p2/6)^0.5 and invp^3 = (p2/6)^-1.5
                p2e = ts(p2, 1e-10, ALU.add)
                lnp2 = act(p2e, AF.Ln, scale=1.0 / 6.0)
                p_ = act(lnp2, AF.Exp, scale=0.5)
                ip3 = act(lnp2, AF.Exp, scale=-1.5)

                # batched determinants of W0=R and W1=B (= A - q I)
                Dw = new([P, 2, 3, 3, Th])
                gp.tensor_copy(Dw[:, 0], Rv)
                gp.tensor_copy(Dw[:, 1], Av)
                Dw_diag = Dw.rearrange("p w a b t -> p w (a b) t")[:, 1, ::4]
                gp.tensor_tensor(out=Dw_diag, in0=Adiag, in1=q_b3,
                                 op=ALU.subtract)

                def dsl(i, j):
                    return Dw[:, :, i, j]                            # [P,2,Th]

                u1 = tt(dsl(1, 1), dsl(2, 2), ALU.mult, eng=gp)
                u2 = tt(dsl(1, 2), dsl(2, 1), ALU.mult, eng=gp)
                cof0 = tt(dsl(0, 0), tt(u1, u2, ALU.subtract, eng=gp),
                          ALU.mult, eng=gp)
                u3 = tt(dsl(1, 0), dsl(2, 2), ALU.mult, eng=gp)
                u4 = tt(dsl(1, 2), dsl(2, 0), ALU.mult, eng=gp)
                cof1 = tt(dsl(0, 1), tt(u3, u4, ALU.subtract, eng=gp),
                          ALU.mult, eng=gp)
                u5 = tt(dsl(1, 0), dsl(2, 1), ALU.mult, eng=gp)
                u6 = tt(dsl(1, 1), dsl(2, 0), ALU.mult, eng=gp)
                cof2 = tt(dsl(0, 2), tt(u5, u6, ALU.subtract, eng=gp),
                          ALU.mult, eng=gp)
                dets = tt(tt(cof0, cof1, ALU.subtract, eng=gp), cof2,
                          ALU.add, eng=gp)
                detR = dets[:, 0]
                detB = dets[:, 1]

                # r = clamp(0.5 * detB * invp^3, -1, 1)
                rr = tt(detB, ip3, ALU.mult, shape=[P, Th])
                r_ = ts(rr, 0.5, ALU.mult, 1.0, ALU.min)
                r_ = ts(r_, -1.0, ALU.max)

                # acos(r) = pi/2 + sign(r) * (poly(|r|)*sqrt(1-|r|) - pi/2)
                tabs = act(r_, AF.Abs)
                poly = poly_eval(tabs, [-0.0187293, 0.0742610,
                                        -0.2121144, 1.5707288])
                u_ = ts(tabs, -1.0, ALU.mult, 1.0, ALU.add)
                sq1mt = act(act(u_, AF.Ln), AF.Exp, scale=0.5)
                sgn = act(r_, AF.Sign)
                pq = tt(poly, sq1mt, ALU.mult)
                inner = ts(pq, -math.pi / 2.0, ALU.add)
                sm = tt(sgn, inner, ALU.mult)
                phi = ts(sm, 1.0 / 3.0, ALU.mult, math.pi / 6.0, ALU.add)

                # cos/sin Taylor on [0,pi/3]; cos(phi+2pi/3) = -.5c - (v3/2)s
                z = tt(phi, phi, ALU.mult)
                cosp = poly_eval(z, [-1.0 / 720, 1.0 / 24, -0.5, 1.0])
                sinp = poly_eval(z, [1.0 / 120, -1.0 / 6, 1.0])
                sinp = tt(sinp, phi, ALU.mult)
                halfc = ts(cosp, -0.5, ALU.mult)
                cosp2 = stt(sinp, -math.sqrt(3.0) / 2.0, halfc,
                            ALU.mult, ALU.add)

                twop = ts(p_, 2.0, ALU.mult)
                eigs = new([P, 3, Th])
                e1t = tt(twop, cosp, ALU.mult)
                nc.vector.tensor_tensor(out=eigs[:, 0], in0=e1t, in1=q,
                                        op=ALU.add)
                e3t = tt(twop, cosp2, ALU.mult)
                nc.vector.tensor_tensor(out=eigs[:, 2], in0=e3t, in1=q,
                                        op=ALU.add)
                q3 = ts(q, 3.0, ALU.mult)
                e12 = tt(eigs[:, 0], eigs[:, 2], ALU.add)
                nc.vector.tensor_tensor(out=eigs[:, 1], in0=q3, in1=e12,
                                        op=ALU.subtract)

                eig_c = ts(eigs.rearrange("p k t -> p (k t)"), 0.0, ALU.max,
                           1e-30, ALU.add)                           # [P,3Th]
                sv = act(act(eig_c, AF.Ln), AF.Exp, scale=0.5)
                sv = sv.rearrange("p (k t) -> p k t", k=3)

                dsgn = act(detR, AF.Sign)
                s12 = tt(sv[:, 0], sv[:, 1], ALU.add)
                ds3 = tt(dsgn, sv[:, 2], ALU.mult)
                trace = tt(s12, ds3, ALU.add)                        # [P,Th]

                e_ = stt(trace, -2.0, e0, ALU.mult, ALU.add)
                e_ = ts(e_, 0.0, ALU.max)
                arg = tt(e_, invn_h, ALU.mult)
                arg = ts(arg, 1e-7, ALU.add)
                y0 = act(act(arg, AF.Ln), AF.Exp, scale=0.5)
                ry = recip(y0)
                ay = tt(arg, ry, ALU.mult)
                outv = ts(tt(y0, ay, ALU.add), 0.5, ALU.mult)

                nc.sync.dma_start(out=outd[:, Th * h:Th * (h + 1)], in_=outv)

    nc.compile()

    # collapse redundant ACT table loads (all funcs used live in
    # natural_log_exp_and_others; retarget + drop dupes, keeping syncs)
    tables = list(get_activation_tables(nc.m.arch).keys())
    target = tables.index("natural_log_exp_and_others")
    for blk in nc.main_func.blocks:
        seen = False
        drop = []
        for inst in list(blk.instructions):
            if isinstance(inst, mybir.InstLoadActFuncSet):
                inst.act_func_set_id = target
                si = inst.sync_info
                has_sync = si is not None and (si.on_wait or si.on_update)
                if seen and not has_sync:
                    drop.append(inst)
                    continue
                seen = True
        for inst in drop:
            blk.instructions.remove(inst)
    return nc


def get_nc_for(num_atoms):
    _, chunks, nmask = _plan(num_atoms)
    key = (chunks, nmask)
    if key not in _CACHE:
        _CACHE[key] = _build(key)
    return _CACHE[key]


def _prep_all(X, Y, nf):
    """Host prep: sort+deal, pack per-core fp8 streams."""
    import ml_dtypes
    fp8 = ml_dtypes.float8_e4m3
    na = np.asarray(nf).astype(np.int64)
    order, chunks, nmask = _plan(na)
    gw = [112 * c + 16 for c in chunks]
    goff = np.concatenate([[0], np.cumsum(gw)]).astype(int)
    W_TOT = int(goff[-1])
    moff = np.concatenate([[0], np.cumsum([16 * m for m in nmask])]).astype(int)
    MK_TOT = max(int(moff[-1]), 16)

    cols = order.reshape(S, NCORES)           # [position, core] -> orig idx
    in_maps = []
    for c in range(NCORES):
        idx = cols[:, c]
        n_c = na[idx].astype(np.float32)
        V = np.zeros((S, M, 7), np.float32)
        V[:, :, 0:3] = X[idx].reshape(S, M, 3)
        V[:, :, 3:6] = Y[idx].reshape(S, M, 3)
        V[:, :, 6] = 1.0
        xz = np.zeros((P, W_TOT), np.float32)
        msk = np.zeros((P, MK_TOT), np.float32)
        pa = np.arange(P, dtype=np.float32)
        for g in range(G):
            ch, nm = chunks[g], nmask[g]
            blk = V[16 * g:16 * g + 16, 0:128 * ch].reshape(
                16, ch, P, 7).transpose(2, 1, 0, 3).reshape(P, ch * W)
            xz[:, goff[g]:goff[g] + ch * W] = blk
            if nm:
                cs = np.arange(ch - nm, ch, dtype=np.float32)
                mg = ((128.0 * cs[None, :, None] + pa[:, None, None])
                      < n_c[16 * g:16 * g + 16][None, None, :])
                msk[:, moff[g]:moff[g + 1]] = mg.reshape(P, 16 * nm)
        invn = (np.float32(1.0) / n_c).astype(np.float32)
        consts = np.concatenate(
            [n_c.reshape(T, P).T, invn.reshape(T, P).T], axis=1
        ).astype(np.float32)
        in_maps.append({"xz": xz.astype(fp8), "msk": msk.astype(fp8),
                        "consts": np.ascontiguousarray(consts)})
    return in_maps, order


def kernel(input, target, num_atoms):
    from concourse.bass_utils import run_bass_kernel_spmd

    X = np.asarray(input, dtype=np.float32)
    Y = np.asarray(target, dtype=np.float32)
    B = X.shape[0]
    assert B == NCORES * S, f"unexpected batch {B}"

    nc = get_nc_for(num_atoms)
    in_maps, order = _prep_all(X, Y, num_atoms)
    res = run_bass_kernel_spmd(nc, in_maps, list(range(NCORES))).results
    out = np.empty(B, np.float32)
    cols = order.reshape(S, NCORES)
    for c in range(NCORES):
        out[cols[:, c]] = res[c]["out"].T.reshape(S)  # [p,t] -> pos 128t+p
    return out


# revision 13
# speedup vs baseline: 1.0323x; 1.0323x over previous
"""Trainium2 Bass kernel for nn_Coords2RMSD (masked Kabsch RMSD loss).

Pure data parallel over 8 NeuronCores. Samples are globally sorted by
num_atoms and dealt round-robin to cores, so all cores share one
compiled schedule: per group of 16 samples, only ceil(max_n/128) atom
chunks are streamed/contracted, and at most the last chunk or two need
a per-sample mask. All 17 per-sample reductions (3x3 correlation,
component sums, sums of squares) are computed on the TensorEngine as a
batched 7x7 Gram matrix in fp8e4m3 (tolerance allows it; fp32 PSUM
accumulate): Z = [X1 X2 X3 Y1 Y2 Y3 one] columns, G = Z^T (mask*Z),
masking only the moving operand (the mask is idempotent). The whole
input stream is DMA'd up-front into persistent SBUF tiles across two
HWDGE rings so the TensorEngine free-runs. Per-sample 7x7 diagonal
blocks are gathered sample-major via a through-DRAM DMA shuffle, and a
closed-form 3x3 eigenvalue epilogue (trig method) turns the reductions
into the RMSD; extraction + epilogue run in two halves so the first
half hides under the main loop.
"""
import math
import numpy as np

P = 128          # partitions
M = 768          # max atoms
NCORES = 8
T = 8            # epilogue tiles (sample p of tile t is position 128t+p)
S = 1024         # samples per core
G = 64           # sample groups per core (16 samples each)
W = 112          # matmul columns per group (16 samples x 7 comps)
NSUP = 16        # xz stream load chunks (4 groups each)

_CACHE = {}


def _plan(na):
    """Global sort + deal; schedule shared by all cores."""
    na = np.asarray(na).astype(np.int64)
    order = np.argsort(na, kind="stable")
    n_pos = na[order].reshape(S, NCORES)      # [position, core]
    ngrp = n_pos.reshape(G, 16, NCORES)
    gmax = ngrp.max(axis=(1, 2))
    gmin = ngrp.min(axis=(1, 2))
    chunks = np.ceil(gmax / 128.0).astype(int)
    cmin = np.minimum(np.floor(gmin / 128.0).astype(int), chunks)
    nmask = chunks - cmin
    return order, tuple(int(c) for c in chunks), tuple(int(m) for m in nmask)


def _build(schedule):
    import concourse.bacc as bacc
    import concourse.mybir as mybir
    from concourse.tile import TileContext
    from concourse.hw_specs import get_activation_tables

    f32 = mybir.dt.float32
    fp8 = mybir.dt.float8e4
    ALU = mybir.AluOpType
    AF = mybir.ActivationFunctionType

    chunks, nmask = schedule
    Tn = T
    gw = [112 * c + 16 for c in chunks]          # stream width per group
    goff = np.concatenate([[0], np.cumsum(gw)]).astype(int)
    W_TOT = int(goff[-1])
    moff = np.concatenate([[0], np.cumsum([16 * m for m in nmask])]).astype(int)
    MK_TOT = max(int(moff[-1]), 16)

    nc = bacc.Bacc()
    xzd = nc.declare_dram_parameter("xz", [P, W_TOT], fp8, isOutput=False)
    mskd = nc.declare_dram_parameter("msk", [P, MK_TOT], fp8, isOutput=False)
    constsd = nc.declare_dram_parameter("consts", [P, 2 * Tn], f32,
                                        isOutput=False)
    outd = nc.declare_dram_parameter("out", [P, Tn], f32, isOutput=True)
    # per-half scratch for the diagonal-block gather: [s(16), g(32), ab(49)]
    scrd = [nc.dram_tensor(f"scr{h}", [16, G // 2, 49], f32, kind="Internal")
            for h in range(2)]

    with TileContext(nc) as tc:
        with tc.tile_pool(name="big", bufs=1) as big, \
             tc.tile_pool(name="wk", bufs=8) as wk, \
             tc.tile_pool(name="ps", bufs=8, space="PSUM") as ps, \
             tc.tile_pool(name="st", bufs=1) as st:
            # whole xz stream resident in SBUF; alternate HWDGE rings.
            # First two supers issue before anything else so PE starts early.
            xz_sup = []
            sup_bounds = []
            gps = G // NSUP
            consts_t = big.tile([P, 2 * Tn], f32)
            msk_t = big.tile([P, MK_TOT], fp8)
            for j in range(NSUP):
                lo, hi = int(goff[4 * j]), int(goff[4 * (j + 1)])
                xt = big.tile([P, hi - lo], fp8, tag=f"xz{j}")
                eng = nc.sync if j % 2 == 0 else nc.scalar
                eng.dma_start(out=xt[:], in_=xzd[:, lo:hi])
                xz_sup.append(xt)
                sup_bounds.append(lo)
                if j == 1:
                    nc.scalar.dma_start(out=msk_t[:], in_=mskd[:])
                    nc.sync.dma_start(out=consts_t[:], in_=constsd[:])
            invn_t = consts_t[:, Tn:2 * Tn]

            # Gram results per half: [112, 32*112] f32
            E_h = [big.tile([W, (G // 2) * W], f32, tag=f"ea{h}", name=f"ea{h}")
                   for h in range(2)]

            for g in range(G):
                ch, nm = chunks[g], nmask[g]
                cmin = ch - nm
                j = g // gps
                base = int(goff[g]) - sup_bounds[j]
                xg = xz_sup[j][:, base:base + gw[g]]

                rm = None
                if nm > 0:
                    rm = wk.tile([P, 112 * nm], fp8, tag="rm")
                    mg = msk_t[:, int(moff[g]):int(moff[g]) + 16 * nm]
                    nc.vector.tensor_tensor(
                        out=rm[:].rearrange("p (c s k) -> p c s k", c=nm, s=16),
                        in0=xg[:, 112 * cmin:112 * ch].rearrange(
                            "p (c s k) -> p c s k", c=nm, s=16),
                        in1=mg.rearrange("p (c s) -> p c s", c=nm)
                            .unsqueeze(3).broadcast_to([P, nm, 16, 7]),
                        op=ALU.mult)

                psum_t = ps.tile([P, W], f32, tag="ps")
                for c in range(ch):
                    # lhsT padded to 128 cols (pad cols only write psum
                    # rows 112-127, never read; host zeros the tail pad)
                    lhs = xg[:, W * c:W * c + 128]
                    if c < cmin:
                        rhs = xg[:, W * c:W * c + W]
                    else:
                        rhs = rm[:, W * (c - cmin):W * (c - cmin) + W]
                    nc.tensor.matmul(psum_t[:], lhsT=lhs, rhs=rhs,
                                     start=(c == 0), stop=(c == ch - 1))

                gh = g % (G // 2)
                nc.scalar.activation(out=E_h[g // (G // 2)][:, W * gh:W * (gh + 1)],
                                     in_=psum_t[0:W, :], func=AF.Copy)

            # ---------------- extraction + epilogue, per half ----------
            cnt = [0]

            def new(shape, nfree=None):
                cnt[0] += 1
                free = int(np.prod(shape[1:]))
                r = st.tile([P, free], f32, tag=f"e{cnt[0]}")
                ap = r[:]
                if len(shape) > 2:
                    names = " ".join(f"d{i}" for i in range(len(shape) - 1))
                    ap = ap.rearrange(f"p ({names}) -> p {names}",
                                      **{f"d{i}": int(shape[1 + i])
                                         for i in range(len(shape) - 1)})
                return ap

            def tt(a, b, op, shape=None, eng=None):
                r = new(list(shape or a.shape))
                (eng or nc.vector).tensor_tensor(out=r, in0=a, in1=b, op=op)
                return r

            def ts(a, s1, op0, s2=None, op1=None, eng=None):
                r = new(list(a.shape))
                if op1 is None:
                    (eng or nc.vector).tensor_scalar(out=r, in0=a, scalar1=s1,
                                                     scalar2=None, op0=op0)
                else:
                    (eng or nc.vector).tensor_scalar(out=r, in0=a, scalar1=s1,
                                                     scalar2=s2, op0=op0,
                                                     op1=op1)
                return r

            def stt(a, s, b, op0, op1, eng=None):
                r = new(list(a.shape))
                (eng or nc.vector).scalar_tensor_tensor(out=r, in0=a, scalar=s,
                                                        in1=b, op0=op0, op1=op1)
                return r

            def act(a, func, scale=1.0, bias=0.0):
                r = new(list(a.shape))
                nc.scalar.activation(out=r, in_=a, func=func,
                                     scale=scale, bias=bias)
                return r

            def aff(a, scale, bias=0.0):
                r = new(list(a.shape))
                nc.scalar.activation(out=r, in_=a, func=AF.Identity,
                                     scale=scale, bias=bias)
                return r

            def recip(a):
                r = new(list(a.shape))
                nc.vector.reciprocal(out=r, in_=a)
                return r

            def red_inner(a, n_keep, eng=None):
                r = new([P, n_keep])
                (eng or nc.vector).tensor_reduce(out=r, in_=a,
                                                 axis=mybir.AxisListType.X,
                                                 op=ALU.add)
                return r

            def poly_eval(x, coeffs):
                g_ = ts(x, coeffs[0], ALU.mult)
                for c in coeffs[1:-1]:
                    g_ = stt(g_, c, x, ALU.add, ALU.mult)
                return ts(g_, coeffs[-1], ALU.add)

            for h in range(2):
                Th = Tn // 2
                # hop B: E_h[7s+a, 112g + 7s+b] -> scr[s, g, 7a+b]
                for s in range(16):
                    src = E_h[h][7 * s:7 * s + 7, :].rearrange(
                        "p (g c) -> p g c", g=G // 2)[:, :, 7 * s:7 * s + 7]
                    dst = scrd[h][s].rearrange("g (a b) -> a g b", a=7)
                    eng = nc.sync if (h == 0 or s % 2 == 0) else nc.scalar
                    eng.dma_start(out=dst, in_=src)
                # hop C: scr[s, 8t+g8, ab] -> ep[16 g8 + s, 49 t + ab]
                ep = big.tile([P, Th * 49], f32, tag=f"ep{h}")
                nc.sync.dma_start(
                    out=ep[:],
                    in_=scrd[h][:].rearrange("s (t g8) ab -> g8 s t ab", t=Th))

                ep_r = ep[:].rearrange("p (t a b) -> p a b t", t=Th, a=7, b=7)
                mmv = ep_r[:, 0:3, 3:6, :]      # [P, i, j, Th] = sum Xi*Yj
                sxv = ep_r[:, 0:3, 6, :]
                syv = ep_r[:, 3:6, 6, :]
                ep_v = ep[:].rearrange("p (t e) -> p t e", t=Th)
                invn_h = invn_t[:, Th * h:Th * (h + 1)]

                # ssx = sum_i G[i,i], ssy = sum_j G[3+j,3+j]
                ssx = red_inner(ep_v[:, :, 0:17:8], Th)
                ssy = red_inner(ep_v[:, :, 24:41:8], Th)

                invn_b3 = invn_h.unsqueeze(1).broadcast_to([P, 3, Th])

                # R_ij = m_ij - (sx_i * invn) * sy_j
                meanx = tt(sxv, invn_b3, ALU.mult)
                meanx_v = meanx.unsqueeze(2).broadcast_to([P, 3, 3, Th])
                sy_v = syv.unsqueeze(1).broadcast_to([P, 3, 3, Th])
                mxsy = tt(meanx_v, sy_v, ALU.mult)
                Rv = tt(mmv, mxsy, ALU.subtract, shape=[P, 3, 3, Th])

                # e0 = ssx + ssy - (|sx|^2 + |sy|^2) * invn  (GpSimd)
                gp = nc.gpsimd
                sx2 = tt(sxv, sxv, ALU.mult, shape=[P, 3, Th], eng=gp)
                sy2 = tt(syv, syv, ALU.mult, shape=[P, 3, Th], eng=gp)
                nrm = tt(sx2, sy2, ALU.add, eng=gp)
                nrms = red_inner(nrm.rearrange("p i t -> p t i"), Th)
                ss = tt(ssx, ssy, ALU.add, eng=gp)
                nrmi = tt(nrms, invn_h, ALU.mult, eng=gp)
                e0 = tt(ss, nrmi, ALU.subtract, eng=gp)              # [P,Th]

                # A = R^T R: one big outer-product then reduce over k
                prods = tt(Rv.unsqueeze(3).broadcast_to([P, 3, 3, 3, Th]),
                           Rv.unsqueeze(2).broadcast_to([P, 3, 3, 3, Th]),
                           ALU.mult, shape=[P, 3, 3, 3, Th])  # [p,k,a,b,t]
                Av = new([P, 3, 3, Th])
                nc.vector.tensor_reduce(
                    out=Av, in_=prods.rearrange("p k a b t -> p a b t k"),
                    axis=mybir.AxisListType.X, op=ALU.add)
                Aflat = Av.rearrange("p a b t -> p (a b) t")
                Adiag = Aflat[:, ::4]                                # [P,3,Th]

                q = aff(red_inner(Adiag.rearrange("p a t -> p t a"), Th),
                        1.0 / 3.0)                                   # [P,Th]
                q_b3 = q.unsqueeze(1).broadcast_to([P, 3, Th])

                # p2 = sum(B^2) = sum(A^2) - 3 q^2   (B = A - q I, tr A = 3q)
                asq = tt(Aflat, Aflat, ALU.mult)
                allsq = red_inner(asq.rearrange("p a t -> p t a"), Th)
                qsq = tt(q, q, ALU.mult)
                p2 = stt(qsq, -3.0, allsq, ALU.mult, ALU.add)        # [P,Th]

                # log-space: p = (p2/6)^0.5 and invp^3 = (p2/6)^-1.5
                p2e = ts(p2, 1e-10, ALU.add)
                lnp2 = act(p2e, AF.Ln, scale=1.0 / 6.0)
                p_ = act(lnp2, AF.Exp, scale=0.5)
                ip3 = act(lnp2, AF.Exp, scale=-1.5)

                # batched determinants of W0=R and W1=B (= A - q I)
                # dets of W0=R, W1=B (= A - qI) via det = row0 . (row1 x
                # row2), with columns duplicated so the rolls are strided
                Dw = new([P, 2, 3, 6, Th])     # [p, w, row, col(dup), t]
                gp.tensor_copy(
                    Dw[:, 0].rearrange("p r (h c) t -> p r h c t", h=2),
                    Rv.unsqueeze(2).broadcast_to([P, 3, 2, 3, Th]))
                gp.tensor_copy(
                    Dw[:, 1].rearrange("p r (h c) t -> p r h c t", h=2),
                    Av.unsqueeze(2).broadcast_to([P, 3, 2, 3, Th]))
                # subtract q on B's diagonal entries (flat col 7r + 3h)
                Bflat = Dw[:, 1].rearrange("p r c t -> p (r c) t")
                gp.tensor_tensor(out=Bflat[:, 0:15:7], in0=Bflat[:, 0:15:7],
                                 in1=q_b3, op=ALU.subtract)
                gp.tensor_tensor(out=Bflat[:, 3:18:7], in0=Bflat[:, 3:18:7],
                                 in1=q_b3, op=ALU.subtract)
                d0 = Dw[:, :, 0]
                d1 = Dw[:, :, 1]
                d2 = Dw[:, :, 2]
                m1 = tt(d1[:, :, 1:4], d2[:, :, 2:5], ALU.mult, eng=gp)
                m2 = tt(d1[:, :, 2:5], d2[:, :, 1:4], ALU.mult, eng=gp)
                cross = tt(m1, m2, ALU.subtract, eng=gp)
                dp = tt(d0[:, :, 0:3], cross, ALU.mult, eng=gp)
                dets = new([P, 2, Th])
                nc.vector.tensor_reduce(
                    out=dets, in_=dp.rearrange("p w i t -> p w t i"),
                    axis=mybir.AxisListType.X, op=ALU.add)
                detR = dets[:, 0]
                detB = dets[:, 1]

                # r = clamp(0.5 * detB * invp^3, -1, 1)
                rr = tt(detB, ip3, ALU.mult, shape=[P, Th])
                r_ = ts(rr, 0.5, ALU.mult, 1.0, ALU.min)
                r_ = ts(r_, -1.0, ALU.max)

                # acos(r) = pi/2 + sign(r) * (poly(|r|)*sqrt(1-|r|) - pi/2)
                tabs = act(r_, AF.Abs)
                poly = poly_eval(tabs, [-0.0187293, 0.0742610,
                                        -0.2121144, 1.5707288])
                u_ = ts(tabs, -1.0, ALU.mult, 1.0, ALU.add)
                sq1mt = act(act(u_, AF.Ln), AF.Exp, scale=0.5)
                sgn = act(r_, AF.Sign)
                pq = tt(poly, sq1mt, ALU.mult)
                inner = ts(pq, -math.pi / 2.0, ALU.add)
                sm = tt(sgn, inner, ALU.mult)
                phi = ts(sm, 1.0 / 3.0, ALU.mult, math.pi / 6.0, ALU.add)

                # cos/sin Taylor on [0,pi/3]; cos(phi+2pi/3) = -.5c - (v3/2)s
                z = tt(phi, phi, ALU.mult)
                cosp = poly_eval(z, [-1.0 / 720, 1.0 / 24, -0.5, 1.0])
                sinp = poly_eval(z, [1.0 / 120, -1.0 / 6, 1.0])
                sinp = tt(sinp, phi, ALU.mult)
                halfc = aff(cosp, -0.5)
                cosp2 = stt(sinp, -math.sqrt(3.0) / 2.0, halfc,
                            ALU.mult, ALU.add)

                twop = aff(p_, 2.0)
                eigs = new([P, 3, Th])
                e1t = tt(twop, cosp, ALU.mult)
                nc.vector.tensor_tensor(out=eigs[:, 0], in0=e1t, in1=q,
                                        op=ALU.add)
                e3t = tt(twop, cosp2, ALU.mult)
                nc.vector.tensor_tensor(out=eigs[:, 2], in0=e3t, in1=q,
                                        op=ALU.add)
                q3 = aff(q, 3.0)
                e12 = tt(eigs[:, 0], eigs[:, 2], ALU.add)
                nc.vector.tensor_tensor(out=eigs[:, 1], in0=q3, in1=e12,
                                        op=ALU.subtract)

                eig_c = ts(eigs.rearrange("p k t -> p (k t)"), 0.0, ALU.max,
                           1e-30, ALU.add)                           # [P,3Th]
                sv = act(act(eig_c, AF.Ln), AF.Exp, scale=0.5)
                sv = sv.rearrange("p (k t) -> p k t", k=3)

                dsgn = act(detR, AF.Sign)
                s12 = tt(sv[:, 0], sv[:, 1], ALU.add)
                ds3 = tt(dsgn, sv[:, 2], ALU.mult)
                trace = tt(s12, ds3, ALU.add)                        # [P,Th]

                e_ = stt(trace, -2.0, e0, ALU.mult, ALU.add)
                e_ = ts(e_, 0.0, ALU.max)
                arg = tt(e_, invn_h, ALU.mult)
                arg = ts(arg, 1e-7, ALU.add)
                y0 = act(act(arg, AF.Ln), AF.Exp, scale=0.5)
                ry = recip(y0)
                ay = tt(arg, ry, ALU.mult)
                outv = ts(tt(y0, ay, ALU.add), 0.5, ALU.mult)

                nc.sync.dma_start(out=outd[:, Th * h:Th * (h + 1)], in_=outv)

    nc.compile()

    # collapse redundant ACT table loads (all funcs used live in
    # natural_log_exp_and_others; retarget + drop dupes, keeping syncs)
    tables = list(get_activation_tables(nc.m.arch).keys())
    target = tables.index("natural_log_exp_and_others")
    for blk in nc.main_func.blocks:
        seen = False
        drop = []
        for inst in list(blk.instructions):
            if isinstance(inst, mybir.InstLoadActFuncSet):
                inst.act_func_set_id = target
                si = inst.sync_info
                has_sync = si is not None and (si.on_wait or si.on_update)
                if seen and not has_sync:
                    drop.append(inst)
                    continue
                seen = True
        for inst in drop:
            blk.instructions.remove(inst)
    return nc


def get_nc_for(num_atoms):
    _, chunks, nmask = _plan(num_atoms)
    key = (chunks, nmask)
    if key not in _CACHE:
        _CACHE[key] = _build(key)
    return _CACHE[key]


def _prep_all(X, Y, nf):
    """Host prep: sort+deal, pack per-core fp8 streams."""
    import ml_dtypes
    fp8 = ml_dtypes.float8_e4m3
    na = np.asarray(nf).astype(np.int64)
    order, chunks, nmask = _plan(na)
    gw = [112 * c + 16 for c in chunks]
    goff = np.concatenate([[0], np.cumsum(gw)]).astype(int)
    W_TOT = int(goff[-1])
    moff = np.concatenate([[0], np.cumsum([16 * m for m in nmask])]).astype(int)
    MK_TOT = max(int(moff[-1]), 16)

    cols = order.reshape(S, NCORES)           # [position, core] -> orig idx
    in_maps = []
    for c in range(NCORES):
        idx = cols[:, c]
        n_c = na[idx].astype(np.float32)
        V = np.zeros((S, M, 7), np.float32)
        V[:, :, 0:3] = X[idx].reshape(S, M, 3)
        V[:, :, 3:6] = Y[idx].reshape(S, M, 3)
        V[:, :, 6] = 1.0
        xz = np.zeros((P, W_TOT), np.float32)
        msk = np.zeros((P, MK_TOT), np.float32)
        pa = np.arange(P, dtype=np.float32)
        for g in range(G):
            ch, nm = chunks[g], nmask[g]
            blk = V[16 * g:16 * g + 16, 0:128 * ch].reshape(
                16, ch, P, 7).transpose(2, 1, 0, 3).reshape(P, ch * W)
            xz[:, goff[g]:goff[g] + ch * W] = blk
            if nm:
                cs = np.arange(ch - nm, ch, dtype=np.float32)
                mg = ((128.0 * cs[None, :, None] + pa[:, None, None])
                      < n_c[16 * g:16 * g + 16][None, None, :])
                msk[:, moff[g]:moff[g + 1]] = mg.reshape(P, 16 * nm)
        invn = (np.float32(1.0) / n_c).astype(np.float32)
        consts = np.concatenate(
            [n_c.reshape(T, P).T, invn.reshape(T, P).T], axis=1
        ).astype(np.float32)
        in_maps.append({"xz": xz.astype(fp8), "msk": msk.astype(fp8),
                        "consts": np.ascontiguousarray(consts)})
    return in_maps, order


def kernel(input, target, num_atoms):
    from concourse.bass_utils import run_bass_kernel_spmd

    X = np.asarray(input, dtype=np.float32)
    Y = np.asarray(target, dtype=np.float32)
    B = X.shape[0]
    assert B == NCORES * S, f"unexpected batch {B}"

    nc = get_nc_for(num_atoms)
    in_maps, order = _prep_all(X, Y, num_atoms)
    res = run_bass_kernel_spmd(nc, in_maps, list(range(NCORES))).results
    out = np.empty(B, np.float32)
    cols = order.reshape(S, NCORES)
    for c in range(NCORES):
        out[cols[:, c]] = res[c]["out"].T.reshape(S)  # [p,t] -> pos 128t+p
    return out


# revision 14
# speedup vs baseline: 1.0902x; 1.0561x over previous
"""Trainium2 Bass kernel for nn_Coords2RMSD (masked Kabsch RMSD loss).

Pure data parallel over 8 NeuronCores. Samples are globally sorted by
num_atoms and dealt round-robin to cores, so all cores share one
compiled schedule: per group of 16 samples, only ceil(max_n/128) atom
chunks are streamed/contracted, and at most the last chunk or two need
a per-sample mask. All 17 per-sample reductions (3x3 correlation,
component sums, sums of squares) are computed on the TensorEngine as a
batched 7x7 Gram matrix in fp8e4m3 (tolerance allows it; fp32 PSUM
accumulate): Z = [X1 X2 X3 Y1 Y2 Y3 one] columns, G = Z^T (mask*Z),
masking only the moving operand (the mask is idempotent). The whole
input stream is DMA'd up-front into persistent SBUF tiles across two
HWDGE rings so the TensorEngine free-runs. Per-sample 7x7 diagonal
blocks are gathered sample-major via a through-DRAM DMA shuffle, and a
closed-form 3x3 eigenvalue epilogue (trig method) turns the reductions
into the RMSD; extraction + epilogue run in two halves so the first
half hides under the main loop.
"""
import math
import numpy as np

P = 128          # partitions
M = 768          # max atoms
NCORES = 8
T = 8            # epilogue tiles (sample p of tile t is position 128t+p)
S = 1024         # samples per core
G = 64           # sample groups per core (16 samples each)
W = 112          # matmul columns per group (16 samples x 7 comps)
NSUP = 16        # xz stream load chunks (4 groups each)

_CACHE = {}


def _plan(na):
    """Global sort + deal; schedule shared by all cores."""
    na = np.asarray(na).astype(np.int64)
    order = np.argsort(na, kind="stable")
    n_pos = na[order].reshape(S, NCORES)      # [position, core]
    ngrp = n_pos.reshape(G, 16, NCORES)
    gmax = ngrp.max(axis=(1, 2))
    gmin = ngrp.min(axis=(1, 2))
    chunks = np.ceil(gmax / 128.0).astype(int)
    cmin = np.minimum(np.floor(gmin / 128.0).astype(int), chunks)
    nmask = chunks - cmin
    return order, tuple(int(c) for c in chunks), tuple(int(m) for m in nmask)


def _build(schedule):
    import concourse.bacc as bacc
    import concourse.mybir as mybir
    from concourse.tile import TileContext
    from concourse.hw_specs import get_activation_tables

    f32 = mybir.dt.float32
    fp8 = mybir.dt.float8e4
    ALU = mybir.AluOpType
    AF = mybir.ActivationFunctionType

    chunks, nmask = schedule
    Tn = T
    gw = [112 * c + 16 for c in chunks]          # stream width per group
    goff = np.concatenate([[0], np.cumsum(gw)]).astype(int)
    W_TOT = int(goff[-1])
    moff = np.concatenate([[0], np.cumsum([16 * m for m in nmask])]).astype(int)
    MK_TOT = max(int(moff[-1]), 16)

    nc = bacc.Bacc()
    xzd = nc.declare_dram_parameter("xz", [P, W_TOT], fp8, isOutput=False)
    mskd = nc.declare_dram_parameter("msk", [P, MK_TOT], fp8, isOutput=False)
    constsd = nc.declare_dram_parameter("consts", [P, 2 * Tn], f32,
                                        isOutput=False)
    outd = nc.declare_dram_parameter("out", [P, Tn], f32, isOutput=True)
    # per-half scratch for the diagonal-block gather: [s(16), g(32), ab(49)]
    scrd = [nc.dram_tensor(f"scr{h}", [16, G // 2, 42], f32, kind="Internal")
            for h in range(2)]

    with TileContext(nc) as tc:
        with tc.tile_pool(name="big", bufs=1) as big, \
             tc.tile_pool(name="wk", bufs=8) as wk, \
             tc.tile_pool(name="ps", bufs=8, space="PSUM") as ps, \
             tc.tile_pool(name="st", bufs=1) as st:
            # whole xz stream resident in SBUF; alternate HWDGE rings.
            # First two supers issue before anything else so PE starts early.
            xz_sup = []
            sup_bounds = []
            gps = G // NSUP
            consts_t = big.tile([P, 2 * Tn], f32)
            msk_t = big.tile([P, MK_TOT], fp8)
            for j in range(NSUP):
                lo, hi = int(goff[4 * j]), int(goff[4 * (j + 1)])
                xt = big.tile([P, hi - lo], fp8, tag=f"xz{j}")
                eng = nc.sync if j % 2 == 0 else nc.scalar
                eng.dma_start(out=xt[:], in_=xzd[:, lo:hi])
                xz_sup.append(xt)
                sup_bounds.append(lo)
                if j == 1:
                    nc.scalar.dma_start(out=msk_t[:], in_=mskd[:])
                    nc.sync.dma_start(out=consts_t[:], in_=constsd[:])
            invn_t = consts_t[:, Tn:2 * Tn]

            # Gram results per half: [112, 32*112] f32
            E_h = [big.tile([W, (G // 2) * W], f32, tag=f"ea{h}", name=f"ea{h}")
                   for h in range(2)]

            for g in range(G):
                ch, nm = chunks[g], nmask[g]
                cmin = ch - nm
                j = g // gps
                base = int(goff[g]) - sup_bounds[j]
                xg = xz_sup[j][:, base:base + gw[g]]

                rm = None
                if nm > 0:
                    rm = wk.tile([P, 112 * nm], fp8, tag="rm")
                    mg = msk_t[:, int(moff[g]):int(moff[g]) + 16 * nm]
                    nc.vector.tensor_tensor(
                        out=rm[:].rearrange("p (c s k) -> p c s k", c=nm, s=16),
                        in0=xg[:, 112 * cmin:112 * ch].rearrange(
                            "p (c s k) -> p c s k", c=nm, s=16),
                        in1=mg.rearrange("p (c s) -> p c s", c=nm)
                            .unsqueeze(3).broadcast_to([P, nm, 16, 7]),
                        op=ALU.mult)

                psum_t = ps.tile([P, W], f32, tag="ps")
                for c in range(ch):
                    # lhsT padded to 128 cols (pad cols only write psum
                    # rows 112-127, never read; host zeros the tail pad)
                    lhs = xg[:, W * c:W * c + 128]
                    if c < cmin:
                        rhs = xg[:, W * c:W * c + W]
                    else:
                        rhs = rm[:, W * (c - cmin):W * (c - cmin) + W]
                    nc.tensor.matmul(psum_t[:], lhsT=lhs, rhs=rhs,
                                     start=(c == 0), stop=(c == ch - 1))

                gh = g % (G // 2)
                nc.scalar.activation(out=E_h[g // (G // 2)][:, W * gh:W * (gh + 1)],
                                     in_=psum_t[0:W, :], func=AF.Copy)

            # ---------------- extraction + epilogue, per half ----------
            cnt = [0]

            def new(shape, nfree=None):
                cnt[0] += 1
                free = int(np.prod(shape[1:]))
                r = st.tile([P, free], f32, tag=f"e{cnt[0]}")
                ap = r[:]
                if len(shape) > 2:
                    names = " ".join(f"d{i}" for i in range(len(shape) - 1))
                    ap = ap.rearrange(f"p ({names}) -> p {names}",
                                      **{f"d{i}": int(shape[1 + i])
                                         for i in range(len(shape) - 1)})
                return ap

            def tt(a, b, op, shape=None, eng=None):
                r = new(list(shape or a.shape))
                (eng or nc.vector).tensor_tensor(out=r, in0=a, in1=b, op=op)
                return r

            def ts(a, s1, op0, s2=None, op1=None, eng=None):
                r = new(list(a.shape))
                if op1 is None:
                    (eng or nc.vector).tensor_scalar(out=r, in0=a, scalar1=s1,
                                                     scalar2=None, op0=op0)
                else:
                    (eng or nc.vector).tensor_scalar(out=r, in0=a, scalar1=s1,
                                                     scalar2=s2, op0=op0,
                                                     op1=op1)
                return r

            def stt(a, s, b, op0, op1, eng=None):
                r = new(list(a.shape))
                (eng or nc.vector).scalar_tensor_tensor(out=r, in0=a, scalar=s,
                                                        in1=b, op0=op0, op1=op1)
                return r

            def act(a, func, scale=1.0, bias=0.0):
                r = new(list(a.shape))
                nc.scalar.activation(out=r, in_=a, func=func,
                                     scale=scale, bias=bias)
                return r

            def aff(a, scale, bias=0.0):
                r = new(list(a.shape))
                nc.scalar.activation(out=r, in_=a, func=AF.Identity,
                                     scale=scale, bias=bias)
                return r

            def recip(a):
                r = new(list(a.shape))
                nc.vector.reciprocal(out=r, in_=a)
                return r

            def red_inner(a, n_keep, eng=None):
                r = new([P, n_keep])
                (eng or nc.vector).tensor_reduce(out=r, in_=a,
                                                 axis=mybir.AxisListType.X,
                                                 op=ALU.add)
                return r

            def poly_eval(x, coeffs):
                g_ = ts(x, coeffs[0], ALU.mult)
                for c in coeffs[1:-1]:
                    g_ = stt(g_, c, x, ALU.add, ALU.mult)
                return ts(g_, coeffs[-1], ALU.add)

            Th = Tn // 2
            ep_h = []
            for h in range(2):
                # hop B: E_h[7s+a, 112g + 7s+b] -> scr[s, g, 7a+b] (a<6;
                # row 6 of each block is never consumed downstream)
                for s in range(16):
                    src = E_h[h][7 * s:7 * s + 6, :].rearrange(
                        "p (g c) -> p g c", g=G // 2)[:, :, 7 * s:7 * s + 7]
                    dst = scrd[h][s].rearrange("g (a b) -> a g b", a=6)
                    eng = (nc.sync if s % 2 == 0 else
                           (nc.gpsimd if h == 0 else nc.scalar))
                    eng.dma_start(out=dst, in_=src)
                # hop C: scr[s, 4t+g8... g = 8t+g8] -> ep[16 g8 + s, t, a, b]
                ep = big.tile([P, Th * 42], f32, tag=f"ep{h}", name=f"ep{h}")
                nc.sync.dma_start(
                    out=ep[:],
                    in_=scrd[h][:].rearrange("s (t g8) ab -> g8 s t ab", t=Th))
                ep_h.append(ep)

            for h in range(2):
                ep = ep_h[h]
                ep_r = ep[:].rearrange("p (t a b) -> p a b t", t=Th, a=6, b=7)
                mmv = ep_r[:, 0:3, 3:6, :]      # [P, i, j, Th] = sum Xi*Yj
                sxv = ep_r[:, 0:3, 6, :]
                syv = ep_r[:, 3:6, 6, :]
                ep_v = ep[:].rearrange("p (t e) -> p t e", t=Th)
                invn_h = invn_t[:, Th * h:Th * (h + 1)]

                # ssx = sum_i G[i,i], ssy = sum_j G[3+j,3+j]
                ssx = red_inner(ep_v[:, :, 0:17:8], Th)
                ssy = red_inner(ep_v[:, :, 24:41:8], Th)

                invn_b3 = invn_h.unsqueeze(1).broadcast_to([P, 3, Th])

                # R_ij = m_ij - (sx_i * invn) * sy_j
                meanx = tt(sxv, invn_b3, ALU.mult)
                meanx_v = meanx.unsqueeze(2).broadcast_to([P, 3, 3, Th])
                sy_v = syv.unsqueeze(1).broadcast_to([P, 3, 3, Th])
                mxsy = tt(meanx_v, sy_v, ALU.mult)
                Rv = tt(mmv, mxsy, ALU.subtract, shape=[P, 3, 3, Th])

                # e0 = ssx + ssy - (|sx|^2 + |sy|^2) * invn  (GpSimd)
                gp = nc.gpsimd
                sx2 = tt(sxv, sxv, ALU.mult, shape=[P, 3, Th], eng=gp)
                sy2 = tt(syv, syv, ALU.mult, shape=[P, 3, Th], eng=gp)
                nrm = tt(sx2, sy2, ALU.add, eng=gp)
                nrms = red_inner(nrm.rearrange("p i t -> p t i"), Th)
                ss = tt(ssx, ssy, ALU.add, eng=gp)
                nrmi = tt(nrms, invn_h, ALU.mult, eng=gp)
                e0 = tt(ss, nrmi, ALU.subtract, eng=gp)              # [P,Th]

                # A = R^T R: one big outer-product then reduce over k
                prods = tt(Rv.unsqueeze(3).broadcast_to([P, 3, 3, 3, Th]),
                           Rv.unsqueeze(2).broadcast_to([P, 3, 3, 3, Th]),
                           ALU.mult, shape=[P, 3, 3, 3, Th])  # [p,k,a,b,t]
                Av = new([P, 3, 3, Th])
                nc.vector.tensor_reduce(
                    out=Av, in_=prods.rearrange("p k a b t -> p a b t k"),
                    axis=mybir.AxisListType.X, op=ALU.add)
                Aflat = Av.rearrange("p a b t -> p (a b) t")
                Adiag = Aflat[:, ::4]                                # [P,3,Th]

                q = aff(red_inner(Adiag.rearrange("p a t -> p t a"), Th),
                        1.0 / 3.0)                                   # [P,Th]
                q_b3 = q.unsqueeze(1).broadcast_to([P, 3, Th])

                # p2 = sum(B^2) = sum(A^2) - 3 q^2   (B = A - q I, tr A = 3q)
                asq = tt(Aflat, Aflat, ALU.mult)
                allsq = red_inner(asq.rearrange("p a t -> p t a"), Th)
                qsq = tt(q, q, ALU.mult)
                p2 = stt(qsq, -3.0, allsq, ALU.mult, ALU.add)        # [P,Th]

                # log-space: p = (p2/6)^0.5 and invp^3 = (p2/6)^-1.5
                p2e = ts(p2, 1e-10, ALU.add)
                lnp2 = act(p2e, AF.Ln, scale=1.0 / 6.0)
                p_ = act(lnp2, AF.Exp, scale=0.5)
                ip3 = act(lnp2, AF.Exp, scale=-1.5)

                # batched determinants of W0=R and W1=B (= A - q I)
                # dets of W0=R, W1=B (= A - qI) via det = row0 . (row1 x
                # row2), with columns duplicated so the rolls are strided
                Dw = new([P, 2, 3, 6, Th])     # [p, w, row, col(dup), t]
                gp.tensor_copy(
                    Dw[:, 0].rearrange("p r (h c) t -> p r h c t", h=2),
                    Rv.unsqueeze(2).broadcast_to([P, 3, 2, 3, Th]))
                gp.tensor_copy(
                    Dw[:, 1].rearrange("p r (h c) t -> p r h c t", h=2),
                    Av.unsqueeze(2).broadcast_to([P, 3, 2, 3, Th]))
                # subtract q on B's diagonal entries (flat col 7r + 3h)
                Bflat = Dw[:, 1].rearrange("p r c t -> p (r c) t")
                gp.tensor_tensor(out=Bflat[:, 0:15:7], in0=Bflat[:, 0:15:7],
                                 in1=q_b3, op=ALU.subtract)
                gp.tensor_tensor(out=Bflat[:, 3:18:7], in0=Bflat[:, 3:18:7],
                                 in1=q_b3, op=ALU.subtract)
                d0 = Dw[:, :, 0]
                d1 = Dw[:, :, 1]
                d2 = Dw[:, :, 2]
                m1 = tt(d1[:, :, 1:4], d2[:, :, 2:5], ALU.mult, eng=gp)
                m2 = tt(d1[:, :, 2:5], d2[:, :, 1:4], ALU.mult, eng=gp)
                cross = tt(m1, m2, ALU.subtract, eng=gp)
                dp = tt(d0[:, :, 0:3], cross, ALU.mult, eng=gp)
                dets = new([P, 2, Th])
                nc.vector.tensor_reduce(
                    out=dets, in_=dp.rearrange("p w i t -> p w t i"),
                    axis=mybir.AxisListType.X, op=ALU.add)
                detR = dets[:, 0]
                detB = dets[:, 1]

                # r = clamp(0.5 * detB * invp^3, -1, 1)
                rr = tt(detB, ip3, ALU.mult, shape=[P, Th])
                r_ = ts(rr, 0.5, ALU.mult, 1.0, ALU.min)
                r_ = ts(r_, -1.0, ALU.max)

                # acos(r) = pi/2 + sign(r) * (poly(|r|)*sqrt(1-|r|) - pi/2)
                tabs = act(r_, AF.Abs)
                poly = poly_eval(tabs, [-0.0187293, 0.0742610,
                                        -0.2121144, 1.5707288])
                u_ = ts(tabs, -1.0, ALU.mult, 1.0, ALU.add)
                sq1mt = act(act(u_, AF.Ln), AF.Exp, scale=0.5)
                sgn = act(r_, AF.Sign)
                pq = tt(poly, sq1mt, ALU.mult)
                inner = ts(pq, -math.pi / 2.0, ALU.add)
                sm = tt(sgn, inner, ALU.mult)
                phi = ts(sm, 1.0 / 3.0, ALU.mult, math.pi / 6.0, ALU.add)

                # cos/sin Taylor on [0,pi/3]; cos(phi+2pi/3) = -.5c - (v3/2)s
                z = tt(phi, phi, ALU.mult)
                cosp = poly_eval(z, [-1.0 / 720, 1.0 / 24, -0.5, 1.0])
                sinp = poly_eval(z, [1.0 / 120, -1.0 / 6, 1.0])
                sinp = tt(sinp, phi, ALU.mult)
                halfc = aff(cosp, -0.5)
                cosp2 = stt(sinp, -math.sqrt(3.0) / 2.0, halfc,
                            ALU.mult, ALU.add)

                twop = aff(p_, 2.0)
                eigs = new([P, 3, Th])
                e1t = tt(twop, cosp, ALU.mult)
                nc.vector.tensor_tensor(out=eigs[:, 0], in0=e1t, in1=q,
                                        op=ALU.add)
                e3t = tt(twop, cosp2, ALU.mult)
                nc.vector.tensor_tensor(out=eigs[:, 2], in0=e3t, in1=q,
                                        op=ALU.add)
                q3 = aff(q, 3.0)
                e12 = tt(eigs[:, 0], eigs[:, 2], ALU.add)
                nc.vector.tensor_tensor(out=eigs[:, 1], in0=q3, in1=e12,
                                        op=ALU.subtract)

                eig_c = ts(eigs.rearrange("p k t -> p (k t)"), 0.0, ALU.max,
                           1e-30, ALU.add)                           # [P,3Th]
                sv = act(act(eig_c, AF.Ln), AF.Exp, scale=0.5)
                sv = sv.rearrange("p (k t) -> p k t", k=3)

                dsgn = act(detR, AF.Sign)
                s12 = tt(sv[:, 0], sv[:, 1], ALU.add)
                ds3 = tt(dsgn, sv[:, 2], ALU.mult)
                trace = tt(s12, ds3, ALU.add)                        # [P,Th]

                e_ = stt(trace, -2.0, e0, ALU.mult, ALU.add)
                e_ = ts(e_, 0.0, ALU.max)
                arg = tt(e_, invn_h, ALU.mult)
                arg = ts(arg, 1e-7, ALU.add)
                y0 = act(act(arg, AF.Ln), AF.Exp, scale=0.5)
                ry = recip(y0)
                ay = tt(arg, ry, ALU.mult)
                outv = ts(tt(y0, ay, ALU.add), 0.5, ALU.mult)

                nc.sync.dma_start(out=outd[:, Th * h:Th * (h + 1)], in_=outv)

    nc.compile()

    # collapse redundant ACT table loads (all funcs used live in
    # natural_log_exp_and_others; retarget + drop dupes, keeping syncs)
    tables = list(get_activation_tables(nc.m.arch).keys())
    target = tables.index("natural_log_exp_and_others")
    for blk in nc.main_func.blocks:
        seen = False
        drop = []
        for inst in list(blk.instructions):
            if isinstance(inst, mybir.InstLoadActFuncSet):
                inst.act_func_set_id = target
                si = inst.sync_info
                has_sync = si is not None and (si.on_wait or si.on_update)
                if seen and not has_sync:
                    drop.append(inst)
                    continue
                seen = True
        for inst in drop:
            blk.instructions.remove(inst)
    return nc


def get_nc_for(num_atoms):
    _, chunks, nmask = _plan(num_atoms)
    key = (chunks, nmask)
    if key not in _CACHE:
        _CACHE[key] = _build(key)
    return _CACHE[key]


def _prep_all(X, Y, nf):
    """Host prep: sort+deal, pack per-core fp8 streams."""
    import ml_dtypes
    fp8 = ml_dtypes.float8_e4m3
    na = np.asarray(nf).astype(np.int64)
    order, chunks, nmask = _plan(na)
    gw = [112 * c + 16 for c in chunks]
    goff = np.concatenate([[0], np.cumsum(gw)]).astype(int)
    W_TOT = int(goff[-1])
    moff = np.concatenate([[0], np.cumsum([16 * m for m in nmask])]).astype(int)
    MK_TOT = max(int(moff[-1]), 16)

    cols = order.reshape(S, NCORES)           # [position, core] -> orig idx
    in_maps = []
    for c in range(NCORES):
        idx = cols[:, c]
        n_c = na[idx].astype(np.float32)
        V = np.zeros((S, M, 7), np.float32)
        V[:, :, 0:3] = X[idx].reshape(S, M, 3)
        V[:, :, 3:6] = Y[idx].reshape(S, M, 3)
        V[:, :, 6] = 1.0
        xz = np.zeros((P, W_TOT), np.float32)
        msk = np.zeros((P, MK_TOT), np.float32)
        pa = np.arange(P, dtype=np.float32)
        for g in range(G):
            ch, nm = chunks[g], nmask[g]
            blk = V[16 * g:16 * g + 16, 0:128 * ch].reshape(
                16, ch, P, 7).transpose(2, 1, 0, 3).reshape(P, ch * W)
            xz[:, goff[g]:goff[g] + ch * W] = blk
            if nm:
                cs = np.arange(ch - nm, ch, dtype=np.float32)
                mg = ((128.0 * cs[None, :, None] + pa[:, None, None])
                      < n_c[16 * g:16 * g + 16][None, None, :])
                msk[:, moff[g]:moff[g + 1]] = mg.reshape(P, 16 * nm)
        invn = (np.float32(1.0) / n_c).astype(np.float32)
        consts = np.concatenate(
            [n_c.reshape(T, P).T, invn.reshape(T, P).T], axis=1
        ).astype(np.float32)
        in_maps.append({"xz": xz.astype(fp8), "msk": msk.astype(fp8),
                        "consts": np.ascontiguousarray(consts)})
    return in_maps, order


def kernel(input, target, num_atoms):
    from concourse.bass_utils import run_bass_kernel_spmd

    X = np.asarray(input, dtype=np.float32)
    Y = np.asarray(target, dtype=np.float32)
    B = X.shape[0]
    assert B == NCORES * S, f"unexpected batch {B}"

    nc = get_nc_for(num_atoms)
    in_maps, order = _prep_all(X, Y, num_atoms)
    res = run_bass_kernel_spmd(nc, in_maps, list(range(NCORES))).results
    out = np.empty(B, np.float32)
    cols = order.reshape(S, NCORES)
    for c in range(NCORES):
        out[cols[:, c]] = res[c]["out"].T.reshape(S)  # [p,t] -> pos 128t+p
    return out
